# revision 1
# baseline (speedup 1.0000x reference)
"""Conv4d (B=2, Ci=32, Co=64, 16^4 spatial, k=3^4, stride 1, pad 1) on 8
Trainium2 NeuronCores.

Sharding: 8 cores = batch(2) x T-quarters(4). Each core computes
out[64co, 4t, 16d, 16h, 16w] for its (b, t-quarter).

Per-core layout: SBUF x tile [128, 6t*6d*324] where partition group
r in {0..3} holds ci=32 channels of the padded input restricted to the
D-halo window [4r, 4r+6) (plus T halo), planes flattened as 18x18=324.
The 4 partition groups process the 4 output-D-quarters concurrently via
PE row-group tiling (tile_position=(32r, 0)).

Each output (t, d-pair) plane-pair accumulates 81 tap matmuls
(K=32ci, M=64co, N=512=2d*16h*16w) in fp32r (TF32) into one PSUM bank
per row group; epilogue adds bias (DVE/ACT) and DMAs out.
"""
import sys

sys.path.insert(0, "/opt/trn_rl_repo")
import numpy as np

N_CORES = 8
TAPS = [(kt, kd, kh, kw) for kt in range(3) for kd in range(3)
        for kh in range(3) for kw in range(3)]

_NC = None


def _build():
    global _NC
    if _NC is not None:
        return _NC
    import concourse.bacc as bacc
    import concourse.tile as tile
    from concourse import mybir

    f32 = mybir.dt.float32
    f32r = mybir.dt.float32r

    nc = bacc.Bacc("TRN2", debug=False, target_bir_lowering=False,
                   num_devices=N_CORES)
    xq = nc.dram_tensor("xq", [128, 6 * 6 * 324], f32r, kind="ExternalInput")
    wq = nc.dram_tensor("wq", [32, 81 * 64], f32r, kind="ExternalInput")
    bq = nc.dram_tensor("biasq", [64, 1], f32, kind="ExternalInput")
    out = nc.dram_tensor("out", [64, 16384], f32, kind="ExternalOutput")

    with tile.TileContext(nc) as tc:
        with tc.tile_pool(name="xp", bufs=1) as xp, \
             tc.tile_pool(name="wp", bufs=1) as wp, \
             tc.tile_pool(name="op", bufs=6) as op_, \
             tc.tile_pool(name="pp", bufs=8, space="PSUM") as pp:
            xtile = xp.tile([128, 11664], f32r)
            for tf in range(6):
                nc.gpsimd.dma_start(xtile[:, tf * 1944:(tf + 1) * 1944],
                                    xq.ap()[:, tf * 1944:(tf + 1) * 1944])
            # weights replicated into all 4 partition groups straight from
            # the small [32, 5184] DRAM copy (4x 0.66MB reads)
            wtile = wp.tile([128, 5184], f32r)
            for r in range(4):
                nc.gpsimd.dma_start(wtile[32 * r:32 * r + 32, :], wq.ap()[:])
            btile = wp.tile([64, 1], f32)
            nc.gpsimd.dma_start(btile[:], bq.ap()[:])

            xv = xtile.rearrange("p (t d h w) -> p t d h w",
                                 t=6, d=6, h=18, w=18)

            for to in range(4):
                for dp in range(2):
                    ps = [pp.tile([64, 512], f32, tag="ps",
                                  name=f"ps_{to}_{dp}_{r}") for r in range(4)]
                    for i, (kt, kd, kh, kw) in enumerate(TAPS):
                        for r in range(4):
                            rhs = xv[32 * r:32 * r + 32, to + kt,
                                     2 * dp + kd: 2 * dp + kd + 2,
                                     kh:kh + 16, kw:kw + 16]
                            lhsT = wtile[32 * r:32 * r + 32,
                                         i * 64:(i + 1) * 64]
                            nc.tensor.matmul(ps[r][:, :], lhsT, rhs,
                                             start=(i == 0), stop=(i == 80),
                                             tile_position=(32 * r, 0))
                    for r in range(4):
                        o = op_.tile([64, 512], f32, tag="ob",
                                     name=f"o_{to}_{dp}_{r}")
                        if r < 2:
                            nc.vector.tensor_scalar_add(o[:], ps[r][:, :],
                                                        btile[:, 0:1])
                        else:
                            nc.scalar.activation(
                                o[:], ps[r][:, :],
                                mybir.ActivationFunctionType.Identity,
                                bias=btile[:, 0:1])
                        off = to * 4096 + (4 * r + 2 * dp) * 256
                        nc.gpsimd.dma_start(out.ap()[:, off:off + 512], o[:])
    nc.compile()
    _NC = nc
    return nc


def _round_tf32(a):
    b = np.ascontiguousarray(a).view(np.uint32)
    r = (b + np.uint32(0x00000FFF) + ((b >> np.uint32(13)) & np.uint32(1))) \
        & np.uint32(0xFFFFE000)
    return r.view(np.float32)


def _prep_inputs(x, weight, bias):
    x = np.asarray(x, dtype=np.float32)
    weight = np.asarray(weight, dtype=np.float32)
    bias = np.asarray(bias, dtype=np.float32)

    w9 = weight.reshape(64, 32, 81).transpose(2, 1, 0)  # [tap, ci, co]
    warr = np.ascontiguousarray(w9.transpose(1, 0, 2)).reshape(32, 81 * 64)
    wq = _round_tf32(warr)
    bq = bias.reshape(64, 1).astype(np.float32)

    in_maps = []
    for b in range(2):
        xpad = np.pad(x[b], ((0, 0), (1, 1), (1, 1), (1, 1), (1, 1)))
        for tq in range(4):
            xt = xpad[:, 4 * tq:4 * tq + 6]  # [32, 6, 18, 18, 18]
            xqc = np.empty((128, 11664), np.float32)
            for r in range(4):
                xqc[32 * r:32 * r + 32] = \
                    xt[:, :, 4 * r:4 * r + 6].reshape(32, -1)
            in_maps.append({"xq": _round_tf32(xqc), "wq": wq, "biasq": bq})
    return in_maps


def run_spmd(x, weight, bias, trace=False, trace_cores=None, tmpdir=None):
    """Returns (output ndarray, BassKernelResults)."""
    from concourse.bass_utils import run_bass_kernel_spmd
    nc = _build()
    in_maps = _prep_inputs(x, weight, bias)
    res = run_bass_kernel_spmd(nc, in_maps, core_ids=list(range(N_CORES)),
                               trace=trace, trace_cores=trace_cores,
                               tmpdir=tmpdir)
    out = np.empty((2, 64, 16, 16, 16, 16), np.float32)
    for c in range(N_CORES):
        b, tq = c // 4, c % 4
        out[b, :, 4 * tq:4 * tq + 4] = \
            res.results[c]["out"].reshape(64, 4, 16, 16, 16)
    return out, res


def kernel(x, weight, bias):
    out, _ = run_spmd(x, weight, bias)
    return out



# revision 4
# speedup vs baseline: 1.7369x; 1.7369x over previous
"""Conv4d (B=2, Ci=32, Co=64, 16^4 spatial, k=3^4, stride 1, pad 1) on 8
Trainium2 NeuronCores.

Sharding: 8 cores = batch(2) x T-quarters(4). Each core computes
out[64co, 4t, 16d, 16h, 16w] for its (b, t-quarter).

Full-array matmul formulation (K=128, M=128, N=512):
 - K = 4 partition groups x 32ci. Group g stores a d-SHIFTED, stride-2
   subsampling of the padded input: X_g[ci, t', d2, h, w] =
   xpad[ci, t', 2*d2+g, h, w] (d2 in 0..7, h/w padded 18x18).
 - M = 128 = (par, co): PE column par*64+co computes output channel co of
   output plane d = 2*d2 + par. Weight slot (g, par) holds tap kd = g-par
   (zero if out of range) -> one matmul contracts all 3 kd taps for BOTH
   output d-parities at once.
 - One matmul per (t, dq-pair, kt, kh, kw): N = 512 = (d2loc 2, h 16, w 16).
   27 taps accumulate per PSUM bank; 432 matmuls/core total, no
   tile_position needed (full 128x128 array each).
Schedule: 4 waves (one per output t-frame) x 4 psum banks; tap-outer order
kt-major so early matmuls only need the first T-chunk of the input DMA.
PE warmup matmuls on a zeroed tile cover the DMA latency and get HAM to
8/8 before real work. Input DMA split across 2 HWDGE queues (sync/scalar)
+ weights on gpsimd SWDGE; output DMA alternates sync/scalar.
"""
import sys

sys.path.insert(0, "/opt/trn_rl_repo")
import numpy as np

N_CORES = 8
# kt-major: taps j=0..8 only touch input T-chunk t, j=9..17 chunk t+1, ...
TAPS27 = [(kt, kh, kw) for kt in range(3) for kh in range(3) for kw in range(3)]

_NC = None


def _build():
    global _NC
    if _NC is not None:
        return _NC
    import concourse.bacc as bacc
    import concourse.tile as tile
    from concourse import mybir

    f32 = mybir.dt.float32
    f32r = mybir.dt.float32r

    nc = bacc.Bacc("TRN2", debug=False, target_bir_lowering=False,
                   num_devices=N_CORES)
    # [(g,ci)=128, (t'=6, d2=8, h=18, w=18)]
    xq = nc.dram_tensor("xq", [128, 6 * 8 * 18 * 18], f32r,
                        kind="ExternalInput")
    # [(g,ci)=128, (tap=27, par=2, co=64)]
    wq = nc.dram_tensor("wq", [128, 27 * 128], f32r, kind="ExternalInput")
    bq = nc.dram_tensor("biasq", [128, 1], f32, kind="ExternalInput")
    # [(par,co)=128, (t=4, dq=4, d2loc=2, hw=256)]
    out = nc.dram_tensor("out", [128, 8192], f32, kind="ExternalOutput")

    TCH = 8 * 18 * 18  # 2592: one t'-chunk of x per partition

    with tile.TileContext(nc) as tc:
        with tc.tile_pool(name="xp", bufs=1) as xp, \
             tc.tile_pool(name="wp", bufs=1) as wp, \
             tc.tile_pool(name="op", bufs=6) as op_, \
             tc.tile_pool(name="pp", bufs=8, space="PSUM") as pp:
            # weights first on the gpsimd SWDGE queue (kt-chunked so the
            # first real matmuls only wait for the kt=0 slice)
            wtile = wp.tile([128, 27 * 128], f32r)
            for kt in range(3):
                nc.gpsimd.dma_start(wtile[:, kt * 1152:(kt + 1) * 1152],
                                    wq.ap()[:, kt * 1152:(kt + 1) * 1152])
            btile = wp.tile([128, 1], f32)
            nc.gpsimd.dma_start(btile[:], bq.ap()[:])

            # x: 6 T-chunks alternating between the two HWDGE queues
            xtile = xp.tile([128, 6 * TCH], f32r)
            for tch in range(6):
                eng = nc.sync if tch % 2 == 0 else nc.scalar
                eng.dma_start(xtile[:, tch * TCH:(tch + 1) * TCH],
                              xq.ap()[:, tch * TCH:(tch + 1) * TCH])

            # PE warmup: zeroed operands, result never read. Keeps the PE
            # busy during the input DMA so HAM un-throttles to 8/8 by the
            # time real matmuls start.
            junk = xp.tile([128, 640], f32)
            nc.vector.memset(junk[:, :], 0.0)
            wu = pp.tile([128, 512], f32, tag="ps", name="wups")
            N_WU = 4
            for i in range(N_WU):
                nc.tensor.matmul(wu[:, :], junk[:, 0:128], junk[:, 128:640],
                                 start=(i == 0), stop=(i == N_WU - 1))

            xv = xtile.rearrange("p (t d h w) -> p t d h w",
                                 t=6, d=8, h=18, w=18)

            for t in range(4):  # wave = output T frame
                ps = [pp.tile([128, 512], f32, tag="ps",
                              name=f"ps_{t}_{dq}") for dq in range(4)]
                for j, (kt, kh, kw) in enumerate(TAPS27):
                    lhsT = wtile[:, j * 128:(j + 1) * 128]
                    for dq in range(4):
                        rhs = xv[:, t + kt, 2 * dq:2 * dq + 2,
                                 kh:kh + 16, kw:kw + 16]
                        nc.tensor.matmul(ps[dq][:, :], lhsT, rhs,
                                         start=(j == 0), stop=(j == 26))
                for dq in range(4):
                    o = op_.tile([128, 512], f32, tag="ob",
                                 name=f"o_{t}_{dq}")
                    if dq % 2 == 0:
                        nc.vector.tensor_scalar_add(o[:], ps[dq][:, :],
                                                    btile[:, 0:1])
                    else:
                        nc.scalar.activation(
                            o[:], ps[dq][:, :],
                            mybir.ActivationFunctionType.Identity,
                            bias=btile[:, 0:1])
                    col = (t * 4 + dq) * 512
                    eng = nc.sync if dq % 2 == 0 else nc.scalar
                    eng.dma_start(out.ap()[:, col:col + 512], o[:])
    nc.compile()
    _NC = nc
    return nc


def _round_tf32(a):
    b = np.ascontiguousarray(a).view(np.uint32)
    r = (b + np.uint32(0x00000FFF) + ((b >> np.uint32(13)) & np.uint32(1))) \
        & np.uint32(0xFFFFE000)
    return r.view(np.float32)


def _prep_inputs(x, weight, bias):
    x = np.asarray(x, dtype=np.float32)
    weight = np.asarray(weight, dtype=np.float32)
    bias = np.asarray(bias, dtype=np.float32)

    # weights: W[g, ci, j, par, co] = weight[co, ci, kt, g-par, kh, kw]
    W = np.zeros((4, 32, 27, 2, 64), np.float32)
    wt = weight.transpose(2, 4, 5, 3, 1, 0)  # [kt, kh, kw, kd, ci, co]
    wt27 = wt.reshape(27, 3, 32, 64)         # [j, kd, ci, co]
    for par in range(2):
        for kd in range(3):
            W[kd + par, :, :, par, :] = wt27[:, kd].transpose(1, 0, 2)
    wqa = _round_tf32(np.ascontiguousarray(W.reshape(128, 27 * 128)))
    bqa = np.concatenate([bias, bias]).reshape(128, 1).astype(np.float32)

    in_maps = []
    for b in range(2):
        xpad = np.pad(x[b], ((0, 0), (1, 1), (1, 1), (1, 1), (1, 1)))
        for tq in range(4):
            xt = xpad[:, 4 * tq:4 * tq + 6]  # [32, 6, 18, 18, 18]
            xqc = np.empty((4, 32, 6, 8, 18, 18), np.float32)
            for g in range(4):
                xqc[g] = xt[:, :, g:g + 16:2]  # d-planes g, g+2, .., g+14
            in_maps.append({"xq": _round_tf32(xqc.reshape(128, -1)),
                            "wq": wqa, "biasq": bqa})
    return in_maps


def run_spmd(x, weight, bias, trace=False, trace_cores=None, tmpdir=None):
    """Returns (output ndarray, BassKernelResults)."""
    from concourse.bass_utils import run_bass_kernel_spmd
    nc = _build()
    in_maps = _prep_inputs(x, weight, bias)
    res = run_bass_kernel_spmd(nc, in_maps, core_ids=list(range(N_CORES)),
                               trace=trace, trace_cores=trace_cores,
                               tmpdir=tmpdir)
    out = np.empty((2, 64, 16, 16, 16, 16), np.float32)
    for c in range(N_CORES):
        b, tq = c // 4, c % 4
        # [par, co, t, dq, d2loc, h, w] -> d = ((dq*2+d2loc)*2+par)
        arr = res.results[c]["out"].reshape(2, 64, 4, 4, 2, 16, 16)
        arr = arr.transpose(1, 2, 3, 4, 0, 5, 6).reshape(64, 4, 16, 16, 16)
        out[b, :, 4 * tq:4 * tq + 4] = arr
    return out, res


def kernel(x, weight, bias):
    out, _ = run_spmd(x, weight, bias)
    return out
